# revision 27
# baseline (speedup 1.0000x reference)
"""v3: tensor-parallel transformer block on 8 NeuronCores.

Sharding: core c = (g, hh) with batch g = c//4 and head-group hh = c%4
(4 of 16 heads). Each core runs LN1 + Q/K/V for its 4 heads over the full
batch element (T=2048), block-causal attention (keys <= (qb+1)*256 per
query block qb — identical shapes on every core), then a ROW-major Wo
partial. The LN1 residual (gamma1*xb + beta1 + bo) rides the Wo matmul
scaled by 1/4 on every core, so the ReduceScatter over the 4-core batch
group sums attention partials AND reconstructs the replicated residual
exactly once. RS output (bf16, 2 row-chunks overlapped with attention)
gives each core its 512 rows feature-complete; LN2 + full FFN run locally
and the output stays row-sharded (host assembles).
"""
import numpy as np
import ml_dtypes

import concourse.bass as bass
import concourse.mybir as mybir
import concourse.tile as tile
from concourse.vector_clock import ScopedClock
from concourse.bass_utils import run_bass_kernel_spmd
from concourse.masks import make_identity

bf16 = ml_dtypes.bfloat16
f32 = mybir.dt.float32
bt16 = mybir.dt.bfloat16
AF = mybir.ActivationFunctionType
OP = mybir.AluOpType

B, T, C, H, DH, DFF = 2, 2048, 1024, 16, 64, 4096
P = 128
QB = 256            # query block rows
HL = 4              # heads per core
HC = HL * DH        # head feature cols per core (256)
R = 512             # own rows per core after RS
CC = C // P         # 8 feature chunks
MM = DFF // P       # 32 ffn chunks
EPS = 1e-5
GROUPS = [[0, 1, 2, 3], [4, 5, 6, 7]]


# --- walrus single-wait patch (same as v2) ---------------------------------
def _patched_drain_and_barrier(self, tick_clock, wait_clock):
    nc = self.nc
    probe = nc.sync.nop(nofuse=True, hint="tail_wait_probe")
    wait_clock.add_sem_waits(probe.ins, ScopedClock({None: tick_clock.global_clock}))
    si = probe.ins.sync_info
    waits = list(si.on_wait) if si is not None else []
    if si is not None:
        si.on_wait = waits[:1]
    for w in waits[1:]:
        n2 = nc.sync.nop(nofuse=True, hint="tail_wait_split")
        n2.ins.sync_info = mybir.SyncInfo(on_wait=[w], on_update=[])
    nc.sync.drain()
    nc.all_engine_barrier()
    assert self.sems is not None
    popped = nc._tile_sem_poison_stack.pop()
    assert popped is self._sem_poison
    nc.clear_and_free_semaphores(list(self.sems.allocated().values()))
    nc.all_engine_barrier()


tile.TileContext._drain_and_barrier = _patched_drain_and_barrier

_MAX_WAITS = 1
_split_counter = [0]


def _split_sync_waits(nc):
    for fn in nc.m.functions:
        for bb in fn.blocks:
            new_insts = []
            for inst in bb.instructions:
                si = getattr(inst, "sync_info", None)
                if si is not None and si.on_wait and len(si.on_wait) > _MAX_WAITS:
                    waits = list(si.on_wait)
                    keep = waits[-_MAX_WAITS:]
                    excess = waits[:-_MAX_WAITS]
                    for i in range(0, len(excess), _MAX_WAITS):
                        _split_counter[0] += 1
                        nop = mybir.InstNoOp(
                            name=f"I-wsplit-{_split_counter[0]}", ins=[], outs=[])
                        nop.engine = inst.engine
                        nop.sync_info = mybir.SyncInfo(
                            on_wait=excess[i:i + _MAX_WAITS], on_update=[])
                        new_insts.append(nop)
                    si.on_wait = keep
                new_insts.append(inst)
            bb.instructions = new_insts
# ---------------------------------------------------------------------------


class Ctx:
    pass


def _ln_rows(g, xt, xb):
    """Row-major LN stats+normalize of xt [128, C] -> xb bt16 (no gamma/beta)."""
    nc = g.nc
    st = g.stats.tile([P, 2, 6], f32, tag="bnst", name="bnst")
    xv = xt.rearrange("p (s d) -> p s d", s=2)
    for sg in range(2):
        nc.vector.bn_stats(out=st[:, sg, :], in_=xv[:, sg, :])
    mv = g.stats.tile([P, 2], f32, tag="bnmv", name="bnmv")
    nc.vector.bn_aggr(out=mv[:], in_=st[:])
    sq = g.stats.tile([P, 1], f32, tag="bnsq", name="bnsq")
    nc.scalar.activation(out=sq[:], in_=mv[:, 1:2], func=AF.Sqrt,
                         bias=g.eps_sb[:], scale=float(C) / (C - 1))
    rstd = g.stats.tile([P, 1], f32, tag="bnrstd", name="bnrstd")
    nc.vector.reciprocal(rstd[:], sq[:])
    negm = g.stats.tile([P, 1], f32, tag="bnnegm", name="bnnegm")
    nc.vector.tensor_scalar(out=negm[:], in0=mv[:, 0:1], scalar1=rstd[:],
                            scalar2=-1.0, op0=OP.mult, op1=OP.mult)
    nc.scalar.activation(out=xb[:], in_=xt[:], func=AF.Identity,
                         bias=negm[:], scale=rstd[:])


def _phase_ln1_vkq(g):
    """LN1 over the 16 batch-row tiles with V/K/Q projections interleaved
    per 512-row block (kq weights preloaded)."""
    nc, tc = g.nc, g.tc
    with tc.tile_pool(name="xio", bufs=4) as xio:
        for rb in range(4):
            for sub in range(4):
                rt = rb * 4 + sub
                r0 = sub * P
                xt = xio.tile([P, C], f32, tag="xin", name="xin")
                nc.sync.dma_start(xt[:], g.xc[rt * P:(rt + 1) * P, :])
                for th in g.dma_after.pop(rt, []):
                    th()
                xb = xio.tile([P, C], bt16, tag="xbn", name="xbn")
                _ln_rows(g, xt, xb)
                ptb = g.ps.tile([P, CC, P], bt16, tag="ps", name="ptb")
                for c in range(CC):
                    nc.tensor.transpose(ptb[:, c, :], xb[:, c * P:(c + 1) * P],
                                        g.identb[:])
                nc.vector.tensor_copy(out=g.x1T[rb][:, :, r0:r0 + P],
                                      in_=ptb[:])
            for sub in range(4):
                kt = rb * 4 + sub
                r0 = sub * P
                nc.vector.memset(g.vv[kt][:, :, DH:DH + 1], 1.0)
                pv = g.ps.tile([P, 512], f32, tag="ps", name="ps_v")
                for c in range(CC):
                    nc.tensor.matmul(pv[:, :HC], g.x1T[rb][:, c, r0:r0 + P],
                                     g.wvs[:, c, :],
                                     start=(c == 0), stop=(c == CC - 1))
                nc.scalar.copy(
                    out=g.vv[kt][:, :, 0:DH],
                    in_=pv[:, :HC].rearrange("p (h d) -> p h d", h=HL))
            for ci in range(2):
                pk = g.ps.tile([P, 512], f32, tag="ps", name="ps_k")
                for c in range(CC):
                    nc.tensor.matmul(pk[:], g.wkm[ci][:, c, :],
                                     g.x1T[rb][:, c, :],
                                     start=(c == 0), stop=(c == CC - 1))
                nc.vector.tensor_scalar(
                    out=g.kT[ci][rb][:, :], in0=pk[:],
                    scalar1=g.bks[:, ci:ci + 1], scalar2=None, op0=OP.add)
                pq = g.ps.tile([P, 512], f32, tag="ps", name="ps_q")
                for c in range(CC):
                    nc.tensor.matmul(pq[:], g.wqm[ci][:, c, :],
                                     g.x1T[rb][:, c, :],
                                     start=(c == 0), stop=(c == CC - 1))
                nc.vector.tensor_scalar(
                    out=g.qT[ci][:, rb * 512:(rb + 1) * 512], in0=pq[:],
                    scalar1=g.bqs[:, ci:ci + 1], scalar2=None, op0=OP.add)


def _attn_unit(g, qbp, ci, hl):
    """Attention for query blocks {2*qbp, 2*qbp+1}, one (ci, hl) head."""
    nc = g.nc
    h = 2 * ci + hl
    hs = slice(hl * DH, (hl + 1) * DH)
    pav = g.pav.tile([P, 512], f32, tag="pav", name="ps_av")
    for qi in range(2):
        qb = 2 * qbp + qi
        q0 = qb * QB
        nkc = 2 * (qb + 1)
        for kg in range(qb + 1):
            psc = g.psc.tile([P, 2, QB], f32, tag="psc", name="ps_s")
            for k in range(2):
                kc = 2 * kg + k
                rb, k0 = kc // 4, (kc % 4) * P
                nc.tensor.matmul(
                    psc[:, k, :], g.kT[ci][rb][hs, k0:k0 + P],
                    g.qT[ci][hs, q0:q0 + QB],
                    start=True, stop=True, tile_position=(hl * DH, 0))
            aQ = g.apl.tile([P, 2, QB], bt16, tag="aQ", name="aQ")
            nc.scalar.activation(out=aQ[:], in_=psc[:], func=AF.Exp)
            if kg == qb:  # diagonal granule: causal mask
                nc.vector.tensor_mul(aQ[:], aQ[:],
                                     g.mq[:, 2 * qb:2 * qb + 2, :])
            for k in range(2):
                kc = 2 * kg + k
                nc.tensor.matmul(
                    pav[:DH + 1, qi * QB:(qi + 1) * QB],
                    g.vv[kc][:, h, :], aQ[:, k, :],
                    start=(kg == 0 and k == 0),
                    stop=(kc == nkc - 1))
    den = g.stats.tile([1, 512], f32, tag="den", name="den", bufs=2)
    nc.vector.tensor_copy(den[:], pav[DH:DH + 1, :512])
    rr = g.stats.tile([1, 512], bt16, tag="rr", name="rr", bufs=2)
    with nc.allow_low_precision(reason="softmax denom bcast bf16"):
        nc.vector.reciprocal(rr[:], den[:])
    prb = g.ps.tile([P, 512], f32, tag="ps", name="ps_r")
    nc.tensor.matmul(prb[:DH, :512], g.ones64[:], rr[:],
                     start=True, stop=True)
    rbc = g.stats.tile([DH, 512], f32, tag="rbc", name="rbc", bufs=2)
    nc.vector.tensor_copy(rbc[:], prb[:DH, :512])
    nc.vector.tensor_mul(
        out=g.hcat[ci][hs, qbp * 512:(qbp + 1) * 512],
        in0=pav[:DH, :512], in1=rbc[:])


def _wo_qt(g, qt):
    """Row-major Wo partial + (1/4) residual rides, for one 128-row tile."""
    nc = g.nc
    if True:
        qs = slice(qt * P, (qt + 1) * P)
        ql = slice((qt % 4) * P, (qt % 4 + 1) * P)
        po = g.wodr.tile([P, C], bt16, tag="po", name="po")
        for half in range(2):
            pa = g.psc.tile([P, 512], f32, tag="psc", name="ps_wo")
            for ci in range(2):
                nc.tensor.matmul(pa[:], g.hcat[ci][:, qs],
                                 g.wos[:, ci, half * 512:(half + 1) * 512],
                                 start=(ci == 0), stop=False)
            for cl in range(4):
                c = half * 4 + cl
                nc.tensor.matmul(pa[:, cl * P:(cl + 1) * P],
                                 g.x1T[qt // 4][:, c, ql], g.gdiag[c][:],
                                 start=False, stop=False)
            nc.tensor.matmul(pa[:], g.onesq[:, qs],
                             g.bors[:, half * 512:(half + 1) * 512],
                             start=False, stop=True)
            nc.vector.tensor_copy(out=po[:, half * 512:(half + 1) * 512],
                                  in_=pa[:])
        if qt < 8:
            dst = g.part[0][qt * P:(qt + 1) * P, :]
        elif qt < 12:
            dst = g.part[1][(qt - 8) * P:(qt - 7) * P, :]
        else:
            dst = g.part[2][(qt - 12) * P:(qt - 11) * P, :]
        nc.sync.dma_start(dst, po[:])


def _ln2_rt(g, rt):
    """LN2 + gamma2/beta2 for own-row tile rt (128 rows) -> x3Tb cols.
    Own rows: rs0[0:256] then rs1[0:192] then rs2[0:64]."""
    nc = g.nc
    x2row = g.cp.tile([P, C], bt16, tag="x2row", name="x2row")
    if rt < 2:
        nc.sync.dma_start(x2row[:], g.rs[0][rt * P:(rt + 1) * P, :])
    else:
        nc.sync.dma_start(x2row[:], g.rs[rt - 1][:, :])
    x2n = g.cp.tile([P, C], bt16, tag="x2n", name="x2n")
    _ln_rows(g, x2row, x2n)
    ptb3 = g.ps.tile([P, CC, P], bt16, tag="ps", name="ptb3")
    for c in range(CC):
        nc.tensor.transpose(ptb3[:, c, :],
                            x2n[:, c * P:(c + 1) * P], g.identb[:])
    for c in range(CC):
        nc.scalar.activation(
            out=g.x3Tb[c][:, rt * P:(rt + 1) * P],
            in_=ptb3[:, c, :], func=AF.Identity,
            bias=g.be2s[:, c:c + 1], scale=g.g2s[:, c:c + 1])


def _phase_ffn(g):
    nc, tc = g.nc, g.tc
    # LN2 tiles 0-2 are ready early (RS chunks 0/1); rt3 waits on the last
    # RS chunk, so FFN1 runs in two column passes: cols 0:384 immediately,
    # cols 384:512 once rt3 lands. w1 tiles stay resident for the second
    # pass (no re-stream).
    for rt in range(3):
        _ln2_rt(g, rt)
    with tc.tile_pool(name="dp", bufs=1) as dp, \
         tc.tile_pool(name="w1p", bufs=1) as w1p, \
         tc.tile_pool(name="w2p", bufs=3) as w2p:
        h1 = [dp.tile([P, R], bt16, tag=f"h1_{m}", name=f"h1_{m}")
              for m in range(MM)]
        w1ms = []
        for m in range(MM):
            if m < 4:
                w1m = g.w1e[m]
            else:
                w1m = w1p.tile([P, CC, P], bt16, tag=f"w1_{m}", name=f"w1m{m}")
                nc.sync.dma_start(
                    w1m[:],
                    g.w1[:, m * P:(m + 1) * P].rearrange("(c p) f -> p c f", p=P))
            w1ms.append(w1m)
            p1 = g.ps.tile([P, 512], f32, tag="ps", name="ps_f1")
            for c in range(CC):
                nc.tensor.matmul(p1[:, :384], w1m[:, c, :],
                                 g.x3Tb[c][:, :384],
                                 start=(c == 0), stop=(c == CC - 1))
            nc.scalar.activation(out=h1[m][:, :384], in_=p1[:, :384],
                                 func=AF.Gelu,
                                 bias=g.b1s[:, m:m + 1], scale=1.0)
        _ln2_rt(g, 3)
        for m in range(MM):
            p1 = g.ps.tile([P, 512], f32, tag="ps", name="ps_f1b")
            for c in range(CC):
                nc.tensor.matmul(p1[:, :P], w1ms[m][:, c, :],
                                 g.x3Tb[c][:, 384:],
                                 start=(c == 0), stop=(c == CC - 1))
            nc.scalar.activation(out=h1[m][:, 384:], in_=p1[:, :P],
                                 func=AF.Gelu,
                                 bias=g.b1s[:, m:m + 1], scale=1.0)
        for oc in range(CC):
            w2m = w2p.tile([P, MM, P], bt16, tag="w2", name="w2m")
            nc.sync.dma_start(
                w2m[:],
                g.w2[:, oc * P:(oc + 1) * P].rearrange("(k p) f -> p k f", p=P))
            p2 = g.ps.tile([P, 512], f32, tag="ps", name="ps_f2")
            for k in range(MM):
                nc.tensor.matmul(p2[:], w2m[:, k, :], h1[k][:, :],
                                 start=(k == 0), stop=(k == MM - 1))
            ot = g.cp.tile([P, R], f32, tag="otile", name="otile")
            nc.vector.scalar_tensor_tensor(
                out=ot[:], in0=p2[:], scalar=g.b2s[:, oc:oc + 1],
                in1=g.x3Tb[oc][:, :], op0=OP.add, op1=OP.add)
            nc.sync.dma_start(g.out[oc], ot[:])


def build_kernel():
    nc = bass.Bass("TRN2", target_bir_lowering=False, num_devices=8)
    g = Ctx()
    g.nc = nc

    g.xc = nc.dram_tensor("xc", [T, C], f32, kind="ExternalInput").ap()
    g.wq = nc.dram_tensor("wq", [C, HC], bt16, kind="ExternalInput").ap()
    g.wk = nc.dram_tensor("wk", [C, HC], bt16, kind="ExternalInput").ap()
    g.wv = nc.dram_tensor("wv", [C, HC], bt16, kind="ExternalInput").ap()
    g.wo = nc.dram_tensor("wo", [HC, C], bt16, kind="ExternalInput").ap()
    g.w1 = nc.dram_tensor("w1", [C, DFF], bt16, kind="ExternalInput").ap()
    g.w2 = nc.dram_tensor("w2", [DFF, C], bt16, kind="ExternalInput").ap()
    g.masks = nc.dram_tensor("masks", [P, 16, QB], bt16,
                             kind="ExternalInput").ap()
    vecs = {}
    for nm, n in [("bq", 2), ("bk", 2), ("b1", MM), ("b2", CC),
                  ("g1", CC), ("g2", CC), ("be2", CC)]:
        vecs[nm] = nc.dram_tensor(nm, [n, P], f32, kind="ExternalInput").ap()
    borb = nc.dram_tensor("borb", [1, C], bt16, kind="ExternalInput").ap()
    g.part = [nc.dram_tensor("partA", [1024, C], bt16, kind="Internal").ap(),
              nc.dram_tensor("partB", [512, C], bt16, kind="Internal").ap(),
              nc.dram_tensor("partC", [512, C], bt16, kind="Internal").ap()]
    g.rs = [nc.dram_tensor("rs0", [QB, C], bt16, kind="Internal").ap(),
            nc.dram_tensor("rs1", [P, C], bt16, kind="Internal").ap(),
            nc.dram_tensor("rs2", [P, C], bt16, kind="Internal").ap()]
    g.out = nc.dram_tensor("out", [CC, P, R], f32, kind="ExternalOutput").ap()

    with tile.TileContext(nc) as tc:
        g.tc = tc
        with tc.tile_pool(name="setup", bufs=1) as setup, \
             tc.tile_pool(name="stats", bufs=4) as stats, \
             tc.tile_pool(name="ps", bufs=2, space="PSUM") as ps, \
             tc.tile_pool(name="psc", bufs=4, space="PSUM") as psc, \
             tc.tile_pool(name="pav", bufs=2, space="PSUM") as pav, \
             tc.tile_pool(name="hp", bufs=1) as hp:
            g.stats, g.ps, g.psc, g.pav = stats, ps, psc, pav

            identb = setup.tile([P, P], bt16, tag="identb", name="identb")
            make_identity(nc, identb[:])
            g.identb = identb
            g.ones64 = setup.tile([1, DH], bt16, tag="ones64", name="ones64")
            nc.vector.memset(g.ones64[:], 1.0)
            g.onesq = setup.tile([1, T], bt16, tag="onesq", name="onesq")
            nc.vector.memset(g.onesq[:], 1.0)
            g.eps_sb = setup.tile([P, 1], f32, tag="eps", name="eps")
            nc.vector.memset(g.eps_sb[:], EPS)

            g.sb_vec = {}
            g.dma_after = {0: [], 1: [], 3: []}

            def _vec_dma(t, ap_):
                return lambda: nc.sync.dma_start(t[:], ap_.rearrange("c p -> p c"))
            for nm, ap_ in vecs.items():
                n = ap_.shape[0]
                t = setup.tile([P, n], f32, tag=f"vec_{nm}", name=f"vec_{nm}")
                g.dma_after[0 if nm in ("bk", "bq") else 3].append(
                    _vec_dma(t, ap_))
                g.sb_vec[nm] = t
            g.bqs, g.bks = g.sb_vec["bq"], g.sb_vec["bk"]
            g.b1s, g.b2s = g.sb_vec["b1"], g.sb_vec["b2"]
            g.g2s, g.be2s = g.sb_vec["g2"], g.sb_vec["be2"]
            # bor as a single-partition row [1, C] for the ones-ride
            bors = setup.tile([1, C], bt16, tag="bors", name="bors")
            g.dma_after[3].append(lambda: nc.sync.dma_start(bors[:], borb))
            g.bors = bors

            g.mq = setup.tile([P, 16, QB], bt16, tag="mask", name="mask")
            g.dma_after[3].append(lambda: nc.sync.dma_start(g.mq[:], g.masks))

            g.hcat = [hp.tile([P, T], bt16, tag=f"hcat{ci}", name=f"hcat{ci}")
                      for ci in range(2)]

            with tc.tile_pool(name="x3p", bufs=1) as x3p, \
                 tc.tile_pool(name="w1e", bufs=1) as w1e, \
                 tc.tile_pool(name="cp", bufs=2) as cp:
                g.cp = cp
                g.w1e = [w1e.tile([P, CC, P], bt16, tag=f"w1e{m}",
                                  name=f"w1e{m}") for m in range(4)]
                g.x3Tb = [x3p.tile([P, R], bt16, tag=f"x3Tb{c}",
                                   name=f"x3Tb{c}") for c in range(CC)]
                with tc.tile_pool(name="kvp", bufs=1) as kvp, \
                     tc.tile_pool(name="x1bp", bufs=1) as x1bp, \
                     tc.tile_pool(name="wkq", bufs=1) as wkq, \
                     tc.tile_pool(name="apl", bufs=4) as apl, \
                     tc.tile_pool(name="wodr", bufs=3) as wodr, \
                     tc.tile_pool(name="wop", bufs=1) as wop, \
                     tc.tile_pool(name="wvp", bufs=1) as wvp:
                    g.apl, g.wodr = apl, wodr
                    g.x1T = [x1bp.tile([P, CC, 512], bt16, tag=f"x1T{rb}",
                                       name=f"x1T{rb}") for rb in range(4)]
                    g.kT = [[kvp.tile([P, 512], bt16, tag=f"kT{ci}_{rb}",
                                      name=f"kT{ci}_{rb}") for rb in range(4)]
                            for ci in range(2)]
                    g.vv = [kvp.tile([P, HL, DH + 1], bt16, tag=f"vv{kt}",
                                     name=f"vv{kt}") for kt in range(T // P)]
                    g.qT = [kvp.tile([P, T], bt16, tag=f"qT{ci}",
                                     name=f"qT{ci}") for ci in range(2)]
                    g.wkm, g.wqm = [], []
                    for ci in range(2):
                        wkm = wkq.tile([P, CC, P], bt16, tag=f"wkm{ci}",
                                       name=f"wkm{ci}")
                        wqm = wkq.tile([P, CC, P], bt16, tag=f"wqm{ci}",
                                       name=f"wqm{ci}")
                        g.wkm.append(wkm)
                        g.wqm.append(wqm)

                        def _wkq_dma(wkm=wkm, wqm=wqm, ci=ci):
                            nc.sync.dma_start(
                                wkm[:], g.wk[:, ci * P:(ci + 1) * P]
                                .rearrange("(c p) f -> p c f", p=P))
                            nc.sync.dma_start(
                                wqm[:], g.wq[:, ci * P:(ci + 1) * P]
                                .rearrange("(c p) f -> p c f", p=P))
                        g.dma_after[1].append(_wkq_dma)
                    wvs = wvp.tile([P, CC, HC], bt16, tag="wvs", name="wvs")
                    g.dma_after[0].append(
                        lambda: nc.sync.dma_start(
                            wvs[:], g.wv.rearrange("(c p) f -> p c f", p=P)))
                    g.wvs = wvs
                    wos = wop.tile([P, 2, C], bt16, tag="wos", name="wos")
                    g.dma_after[3].append(
                        lambda: nc.sync.dma_start(
                            wos[:], g.wo.rearrange("(ci p) f -> p ci f", p=P)))
                    g.wos = wos

                    _phase_ln1_vkq(g)
                    # 0.25*diag(gamma1) per chunk for the Wo residual rides
                    g.gdiag = []
                    for m in range(CC):
                        gd = setup.tile([P, P], bt16, tag=f"gd{m}",
                                        name=f"gd{m}")
                        nc.vector.tensor_scalar(
                            out=gd[:], in0=g.identb[:],
                            scalar1=g.sb_vec["g1"][:, m:m + 1], scalar2=0.25,
                            op0=OP.mult, op1=OP.mult)
                        g.gdiag.append(gd)
                    def _rs(i):
                        nc.gpsimd.collective_compute(
                            "ReduceScatter", OP.add, replica_groups=GROUPS,
                            ins=[g.part[i]], outs=[g.rs[i]])
                    # Wo tiles of qbp-1 weave between qbp's attention
                    # units so the PE fills the exp-bound bubbles; RS
                    # chunks fire as soon as their Wo rows are complete.
                    units = [(0, 0), (0, 1), (1, 0), (1, 1)]
                    for ci, hl in units:
                        _attn_unit(g, 0, ci, hl)
                    for qbp in (1, 2):
                        for u_i, (ci, hl) in enumerate(units):
                            _attn_unit(g, qbp, ci, hl)
                            _wo_qt(g, 4 * (qbp - 1) + u_i)
                        if qbp == 2:
                            _rs(0)
                            for m in range(4):
                                nc.sync.dma_start(
                                    g.w1e[m][:],
                                    g.w1[:, m * P:(m + 1) * P]
                                    .rearrange("(c p) f -> p c f", p=P))
                    _attn_unit(g, 3, 0, 0)
                    for qt in range(8, 12):
                        _wo_qt(g, qt)
                    _rs(1)
                    for ci, hl in units[1:]:
                        _attn_unit(g, 3, ci, hl)
                    for qt in range(12, 16):
                        _wo_qt(g, qt)
                    _rs(2)
                _phase_ffn(g)
    _split_sync_waits(nc)
    return nc


_NC_CACHE = None


def _get_nc():
    global _NC_CACHE
    if _NC_CACHE is None:
        _NC_CACHE = build_kernel()
    return _NC_CACHE


def _prep_core(inputs, hh):
    """Per-head-group weight slices (head-group hh: heads 4hh..4hh+3)."""
    scale = DH ** -0.5
    Wq = np.asarray(inputs["Wq"], np.float32)
    Wk = np.asarray(inputs["Wk"], np.float32)
    Wv = np.asarray(inputs["Wv"], np.float32)
    Wo = np.asarray(inputs["Wo"], np.float32)
    g1 = np.asarray(inputs["gamma1"], np.float32).reshape(C)
    be1 = np.asarray(inputs["beta1"], np.float32).reshape(C)
    hsl = slice(4 * hh, 4 * hh + 4)
    Wq2 = np.ascontiguousarray(Wq[hsl].transpose(1, 0, 2).reshape(C, HC))
    Wk2 = np.ascontiguousarray(Wk[hsl].transpose(1, 0, 2).reshape(C, HC))
    Wv2 = np.ascontiguousarray(Wv[hsl].transpose(1, 0, 2).reshape(C, HC))
    wq_eff = (g1[:, None] * Wq2) * scale
    wk_eff = g1[:, None] * Wk2
    wv_eff = g1[:, None] * Wv2
    bq_eff = (np.asarray(inputs["bq"], np.float32)[hsl].reshape(HC)
              + be1 @ Wq2) * scale
    bk_eff = np.asarray(inputs["bk"], np.float32)[hsl].reshape(HC) + be1 @ Wk2
    bv_eff = np.asarray(inputs["bv"], np.float32)[hsl].reshape(HC) + be1 @ Wv2
    Wo_sl = Wo[hh * HC:(hh + 1) * HC, :]
    # ride bias: this core's bv@Wo slice plus 1/4 of the replicated terms
    bor = (bv_eff @ Wo_sl
           + 0.25 * (np.asarray(inputs["bo"], np.float32) + be1))
    return {
        "wq": wq_eff.astype(bf16),
        "wk": wk_eff.astype(bf16),
        "wv": wv_eff.astype(bf16),
        "wo": Wo_sl.astype(bf16),
        "bq": bq_eff.reshape(2, P).copy(),
        "bk": bk_eff.reshape(2, P).copy(),
        "borb": bor.reshape(1, C).astype(bf16),
    }


def _diag_masks():
    """[128, 16, 256] bf16: [., 2qb+k, .] = causal mask of diagonal chunk k
    for query block qb (same on every core)."""
    out = np.zeros((P, 16, QB), np.float32)
    for qb in range(8):
        for k in range(2):
            kc = 2 * qb + k
            key = kc * P + np.arange(P)[:, None]
            qglob = qb * QB + np.arange(QB)[None, :]
            out[:, 2 * qb + k, :] = (key <= qglob)
    return out.astype(bf16)


def _make_in_maps(inputs):
    x = np.asarray(inputs["x"], np.float32)
    W1 = np.asarray(inputs["W1"], np.float32).astype(bf16)
    W2 = np.asarray(inputs["W2"], np.float32).astype(bf16)
    g1 = np.asarray(inputs["gamma1"], np.float32).reshape(CC, P)
    shared = {
        "w1": W1, "w2": W2,
        "b1": np.asarray(inputs["b1"], np.float32).reshape(MM, P).copy(),
        "b2": np.asarray(inputs["b2"], np.float32).reshape(CC, P).copy(),
        "g1": g1.copy(),
        "g2": np.asarray(inputs["gamma2"], np.float32).reshape(CC, P).copy(),
        "be2": np.asarray(inputs["beta2"], np.float32).reshape(CC, P).copy(),
        "masks": _diag_masks(),
    }
    per_hh = [_prep_core(inputs, hh) for hh in range(4)]
    in_maps = []
    for c in range(8):
        gg, hh = c // 4, c % 4
        m = dict(shared)
        m.update(per_hh[hh])
        m["xc"] = np.ascontiguousarray(x[gg])
        in_maps.append(m)
    return in_maps


def _assemble(results):
    out = np.zeros((B, T, C), np.float32)
    for c in range(8):
        gg, hh = c // 4, c % 4
        o = results[c]["out"].reshape(C, R).T  # [512, C]
        out[gg, hh * QB:(hh + 1) * QB] = o[:QB]
        out[gg, 1024 + hh * P:1024 + (hh + 1) * P] = o[QB:QB + P]
        out[gg, 1536 + hh * P:1536 + (hh + 1) * P] = o[QB + P:]
    return out


def kernel(**inputs):
    in_maps = _make_in_maps(inputs)
    nc = _get_nc()
    res = run_bass_kernel_spmd(nc, in_maps, core_ids=list(range(8)))
    return _assemble(res.results)
